# revision 1
# baseline (speedup 1.0000x reference)
"""Block-sparse matmul + bias + relu on 8 Trainium2 NeuronCores.

Strategy (data-parallel over batch):
  - Shard x along batch: 8 cores x 512 rows. w_blocks/bias replicated.
  - Per core, compute out^T = sum_blocks w_ij^T-style per-block matmuls with
    the PE in 32x32 tiling mode:
      * x^T resident in SBUF as [128, 32, 512]: input block i lives at
        partitions 32*(i%4) .. 32*(i%4)+31, free tile i//4.
      * each nonzero block (i,j) is one matmul: lhsT = w_block [K=32, M=32],
        rhs = x^T block i [32, 512], accumulated into PSUM at partition strip
        32*(j%4) of bank (i%4): tile_position=(32*(i%4), 32*(j%4)).
      * output block-cols processed in quads (4 cols -> 4 strips x 4 banks),
        16 PE tiles run concurrently.
  - Per quad combine: DVE sums bank pairs, GPSIMD sums the pair results,
    ACT applies bias + relu, DMA out^T tile to DRAM.
  - Host: transpose/cast prep (bf16 feeds the PE; fp32 accumulate in PSUM).
"""

import os

import numpy as np
import ml_dtypes

import concourse.bass as bass
import concourse.tile as tile
from concourse import mybir
from concourse.bass_utils import run_bass_kernel_spmd

LAST_RESULTS = None  # test-only: BassKernelResults of the last run

BS = 32
KB = 128
NB = 128
BATCH = 4096
NCORES = 8
BC = BATCH // NCORES          # 512 batch rows per core
NQ = NB // 4                  # 32 quads of output block-cols
if os.environ.get("BASS_KERNEL_F32R"):
    IN_DT = mybir.dt.float32r
    IN_NP = np.float32
else:
    IN_DT = mybir.dt.bfloat16
    IN_NP = ml_dtypes.bfloat16
if os.environ.get("BASS_KERNEL_OUT_BF16"):
    OUT_DT = mybir.dt.bfloat16
    OUT_NP = ml_dtypes.bfloat16
else:
    OUT_DT = mybir.dt.float32
    OUT_NP = np.float32

_CACHE = {}


def _build_schedule(row_idx, col_idx):
    """Schedule: per quad, round-robin emission over the 16 (strip, rowgrp)
    FIFOs. Returns (sched, S) where sched[q] is a list of
    (r, c, t, slot, start, stop) and S is the per-strip slot count in w image.
    Dummy (zero-weight) entries have slot == -1... they get real slots in the
    zero-padded region; we give them slot index with block = None marker via
    t=0 and a dedicated zero slot per strip.
    """
    nnz = len(row_idx)
    # FIFOs[q][c][r] -> list of block ids
    fifos = [[[[] for _ in range(4)] for _ in range(4)] for _ in range(NQ)]
    for n in range(nnz):
        i = int(row_idx[n]); j = int(col_idx[n])
        fifos[j // 4][j % 4][i % 4].append(n)

    slot_ctr = [0, 0, 0, 0]           # per row-group strip
    sched = []
    slot_of = {}                      # block id -> slot (in its strip)
    dummy_slots = []                  # (r, slot) zero-weight slots
    for q in range(NQ):
        # pad: every (c, r) needs >= 1 entry so PSUM region is defined
        entries = []                  # (r, c, t, block_or_None)
        maxlen = 0
        for c in range(4):
            for r in range(4):
                if not fifos[q][c][r]:
                    fifos[q][c][r].append(None)
                maxlen = max(maxlen, len(fifos[q][c][r]))
        emitted = []
        # r cycles fastest: consecutive MMs hit different row groups so the
        # PE can pull the next LDWEIGHTS ahead of in-flight MATMULs.
        for s in range(maxlen):
            for c in range(4):
                for r in range(4):
                    lst = fifos[q][c][r]
                    if s < len(lst):
                        n = lst[s]
                        if n is None:
                            slot = slot_ctr[r]; slot_ctr[r] += 1
                            dummy_slots.append((r, slot))
                            t = 0
                        else:
                            slot = slot_ctr[r]; slot_ctr[r] += 1
                            slot_of[n] = slot
                            t = int(row_idx[n]) // 4
                        emitted.append([r, c, t, slot, False, False])
        # start/stop flags per (bank r, strip c) accumulation region: the
        # PSUM has_written clear from start=True covers only the partitions
        # the matmul writes (one 32-partition strip, full bank width), so
        # every strip needs its own start.
        first_seen = set()
        for e in emitted:
            key = (e[0], e[1])
            if key not in first_seen:
                e[4] = True
                first_seen.add(key)
        last_idx = {}
        for k, e in enumerate(emitted):
            last_idx[(e[0], e[1])] = k
        for k in last_idx.values():
            emitted[k][5] = True
        sched.append([tuple(e) for e in emitted])
    S = max(slot_ctr)
    return sched, S, slot_of, dummy_slots


def _build_schedule_m128(row_idx, col_idx):
    """M=128 row-mode schedule: one matmul per (input block i, quad q) pair
    covering all four quad columns at once (lhsT [32, 128], zero-padded for
    missing cols). Output writes the full bank (partitions 0..127), so
    tile_position = (32*(i%4), 0).

    Returns (sched, S) with sched[q] = list of (r, t, slot, start, stop);
    slot indexes [32, 128] wide slots in the per-strip weight image, and
    wfill = list of (r, slot, c, n) for the image builder.
    """
    nnz = len(row_idx)
    by_iq = {}
    for n in range(nnz):
        i = int(row_idx[n]); j = int(col_idx[n])
        by_iq.setdefault((j // 4, i), []).append((j % 4, n))

    slot_ctr = [0, 0, 0, 0]
    sched = []
    wfill = []
    for q in range(NQ):
        fifos = [[] for _ in range(4)]        # per row group: list of i (or None)
        for i in range(KB):
            if (q, i) in by_iq:
                fifos[i % 4].append(i)
        for r in range(4):
            if not fifos[r]:
                fifos[r].append(None)
        emitted = []
        maxlen = max(len(f) for f in fifos)
        for s in range(maxlen):
            for r in range(4):
                if s < len(fifos[r]):
                    i = fifos[r][s]
                    slot = slot_ctr[r]; slot_ctr[r] += 1
                    if i is not None:
                        for (c, n) in by_iq[(q, i)]:
                            wfill.append((r, slot, c, n))
                        t = i // 4
                    else:
                        t = 0
                    emitted.append([r, t, slot, False, False])
        first_seen = set()
        for e in emitted:
            if e[0] not in first_seen:
                e[3] = True
                first_seen.add(e[0])
        last_idx = {}
        for k, e in enumerate(emitted):
            last_idx[e[0]] = k
        for k in last_idx.values():
            emitted[k][4] = True
        sched.append([tuple(e) for e in emitted])
    S = max(slot_ctr)
    return sched, S, wfill


_MULTIWAIT_OK = {"InstDMACopy", "InstUnconditionalBranch",
                 "InstConditionalBranch"}


def _legalize_waits(nc):
    """Engine ISA structs carry a single sync-wait slot; Tile can emit more.
    Offload excess waits onto same-engine NoOps inserted just before the
    instruction (per-engine stream order is the block list order)."""
    ctr = 0
    for f in nc.m.functions:
        for blk in f.blocks:
            out = []
            for inst in blk.instructions:
                si = inst.sync_info
                if (si is not None and si.on_wait and len(si.on_wait) > 1
                        and type(inst).__name__ == "InstDMACopy"):
                    # HWDGE lane sems are monotonic add-only counters; a
                    # DMA's wait on its own completion lane orders it against
                    # unrelated prior DMAs on that lane and is droppable.
                    own = {u.ant_name for u in (si.on_update or [])}
                    keep = [w for w in si.on_wait if w.ant_name not in own]
                    if len(keep) > 1:
                        raise RuntimeError(
                            f"DMA {inst.name} still has waits {keep}")
                    inst.sync_info = mybir.SyncInfo(on_wait=keep,
                                                    on_update=si.on_update)
                    out.append(inst)
                    continue
                if (si is not None and si.on_wait and len(si.on_wait) > 1
                        and type(inst).__name__ not in _MULTIWAIT_OK):
                    waits = list(si.on_wait)
                    for w in waits[:-1]:
                        nop = mybir.InstNoOp(name=f"waitnop-{ctr}")
                        ctr += 1
                        nop.engine = inst.engine
                        nop.sync_info = mybir.SyncInfo(on_wait=[w], on_update=[])
                        out.append(nop)
                    inst.sync_info = mybir.SyncInfo(on_wait=[waits[-1]],
                                                    on_update=si.on_update)
                out.append(inst)
            blk.instructions[:] = out


def _build_program(sched, S, repeat=1, loop_n=0, m128=False, dyn_loop=False):
    WSLOT = 128 if m128 else 32
    nc = bass.Bass("TRN2", target_bir_lowering=False, debug=False,
                   num_devices=NCORES)
    x_d = nc.dram_tensor("xt", [128, 32 * BC], IN_DT, kind="ExternalInput").ap()
    w_d = nc.dram_tensor("wim", [128, S * WSLOT], IN_DT, kind="ExternalInput").ap()
    b_d = nc.dram_tensor("bias", [128, 32], mybir.dt.float32,
                         kind="ExternalInput").ap()
    o_d = nc.dram_tensor("outT", [NQ, 128, BC], OUT_DT, kind="ExternalOutput").ap()
    ln_d = None
    if dyn_loop:
        ln_d = nc.dram_tensor("loopn", [1, 1], mybir.dt.uint32,
                              kind="ExternalInput").ap()

    import contextlib

    with tile.TileContext(nc) as tc:
        if dyn_loop:
            tmp = nc.alloc_registers("loopn_tmp", mybir.ALL_ENGINES)
            nc.regs_load(tmp, ln_d[0:1, 0:1])
            loop_end = nc.snap(tmp, donate=True, min_val=0, max_val=1 << 20)
            loop_cm = tc.For_i(0, loop_end, 1)
        elif loop_n:
            loop_cm = tc.For_i(0, loop_n, 1)
        else:
            loop_cm = contextlib.nullcontext()
        with tc.tile_pool(name="const", bufs=1) as cpool, \
             tc.tile_pool(name="work", bufs=3) as wpool, \
             tc.tile_pool(name="psum", bufs=2, space="PSUM") as ppool, \
             loop_cm:
            xt = cpool.tile([128, 32 * BC], IN_DT)
            wt = cpool.tile([128, S * WSLOT], IN_DT)
            bt = cpool.tile([128, 32], mybir.dt.float32)
            nc.sync.dma_start(bt[:], b_d[:])
            # x: chunked DMA (16 x 1MB)
            xch = (32 * BC) // 16
            for k in range(16):
                nc.sync.dma_start(xt[:, k * xch:(k + 1) * xch],
                                  x_d[:, k * xch:(k + 1) * xch])
            # w: chunked DMA in slot order so early quads unblock early
            wch = 8 if not m128 else 16
            wstep = -(-S // wch) * WSLOT
            for k in range(wch):
                lo = k * wstep
                hi = min(S * WSLOT, lo + wstep)
                if lo >= hi:
                    continue
                nc.sync.dma_start(wt[:, lo:hi], w_d[:, lo:hi])

            for rep in range(repeat):
              for q in range(NQ):
                acc = [ppool.tile([128, BC], mybir.dt.float32, tag=f"acc{r}",
                                  name=f"acc{r}_q{q}_p{rep}")
                       for r in range(4)]
                if m128:
                    for (r, t, slot, start, stop) in sched[q]:
                        nc.tensor.matmul(
                            out=acc[r][:, :],
                            lhsT=wt[32 * r:32 * r + 32,
                                    slot * 128:(slot + 1) * 128],
                            rhs=xt[32 * r:32 * r + 32, t * BC:(t + 1) * BC],
                            start=start, stop=stop,
                            tile_position=(32 * r, 0),
                            skip_group_check=True,
                        )
                else:
                    for (r, c, t, slot, start, stop) in sched[q]:
                        nc.tensor.matmul(
                            out=acc[r][32 * c:32 * c + 32, :],
                            lhsT=wt[32 * r:32 * r + 32,
                                    slot * 32:(slot + 1) * 32],
                            rhs=xt[32 * r:32 * r + 32, t * BC:(t + 1) * BC],
                            start=start, stop=stop,
                            tile_position=(32 * r, 32 * c),
                            skip_group_check=True,
                        )
                e0 = wpool.tile([128, BC], mybir.dt.float32, tag="e0")
                e2 = wpool.tile([128, BC], mybir.dt.float32, tag="e2")
                s1 = wpool.tile([128, BC], mybir.dt.float32, tag="s1")
                s2 = wpool.tile([128, BC], mybir.dt.float32, tag="s2")
                s3 = wpool.tile([128, BC], mybir.dt.float32, tag="s3")
                ot = wpool.tile([128, BC], OUT_DT, tag="ot")
                nc.scalar.copy(e0[:], acc[0][:])
                nc.scalar.copy(e2[:], acc[2][:])
                nc.vector.tensor_add(s1[:], acc[1][:], e0[:])
                nc.vector.tensor_add(s2[:], acc[3][:], e2[:])
                nc.gpsimd.tensor_add(s3[:], s1[:], s2[:])
                nc.gpsimd.tensor_scalar(ot[:], s3[:], bt[:, q:q + 1], 0.0,
                                        mybir.AluOpType.add,
                                        mybir.AluOpType.max)
                nc.sync.dma_start(o_d[q], ot[:])
    _legalize_waits(nc)
    return nc


def _prep_inputs_m128(x, w_blocks, bias, row_idx, col_idx, wfill, S):
    xb = x.astype(IN_NP).reshape(BATCH, 32, 4, 32)
    xt_all = np.ascontiguousarray(xb.transpose(2, 3, 1, 0)).reshape(128, 32, BATCH)
    xts = [np.ascontiguousarray(xt_all[:, :, c * BC:(c + 1) * BC]
                                ).reshape(128, 32 * BC) for c in range(NCORES)]
    bim = np.ascontiguousarray(
        bias.astype(np.float32).reshape(32, 4, 32).transpose(1, 2, 0)
    ).reshape(128, 32)
    wim = np.zeros((128, S * 128), dtype=IN_NP)
    wb = w_blocks.astype(IN_NP)
    for (r, slot, c, n) in wfill:
        wim[32 * r:32 * r + 32, 128 * slot + 32 * c:128 * slot + 32 * c + 32] \
            = wb[n]
    return xts, wim, bim


def _prep_inputs(x, w_blocks, bias, row_idx, col_idx, slot_of, dummy_slots, S):
    nnz = len(row_idx)
    # x^T images per core: [128, 32, BC] -> block i at partitions 32*(i%4),
    # free tile i//4.  x[b, 32*(4t+r)+p] -> xt[32r+p, t, b]
    xb = x.astype(IN_NP).reshape(BATCH, 32, 4, 32)        # b, t, r, p
    xt_all = np.ascontiguousarray(xb.transpose(2, 3, 1, 0))  # r, p, t, b
    xt_all = xt_all.reshape(128, 32, BATCH)
    xts = [np.ascontiguousarray(xt_all[:, :, c * BC:(c + 1) * BC]
                                ).reshape(128, 32 * BC) for c in range(NCORES)]
    # w image [128, S*32]
    wim = np.zeros((128, S * 32), dtype=IN_NP)
    wb = w_blocks.astype(IN_NP)
    for n in range(nnz):
        r = int(row_idx[n]) % 4
        s = slot_of[n]
        wim[32 * r:32 * r + 32, 32 * s:32 * s + 32] = wb[n]
    # dummy slots already zero
    bim = np.ascontiguousarray(
        bias.astype(np.float32).reshape(32, 4, 32).transpose(1, 2, 0)
    ).reshape(128, 32)
    return xts, wim, bim


def kernel(x, w_blocks, bias, row_idx, col_idx):
    repeat = int(os.environ.get("BASS_KERNEL_REPEAT", "1"))
    m128 = bool(os.environ.get("BASS_KERNEL_M128"))
    key = (row_idx.tobytes(), col_idx.tobytes(), repeat, m128)
    if key not in _CACHE:
        if m128:
            sched, S, wfill = _build_schedule_m128(row_idx, col_idx)
            aux = wfill
        else:
            sched, S, slot_of, dummy_slots = _build_schedule(row_idx, col_idx)
            aux = (slot_of, dummy_slots)
        nc = _build_program(sched, S, repeat=repeat, m128=m128)
        _CACHE[key] = (nc, S, aux)
    nc, S, aux = _CACHE[key]

    if m128:
        xts, wim, bim = _prep_inputs_m128(x, w_blocks, bias, row_idx, col_idx,
                                          aux, S)
    else:
        slot_of, dummy_slots = aux
        xts, wim, bim = _prep_inputs(x, w_blocks, bias, row_idx, col_idx,
                                     slot_of, dummy_slots, S)
    in_maps = [{"xt": xts[c], "wim": wim, "bias": bim} for c in range(NCORES)]
    trace = bool(os.environ.get("BASS_KERNEL_TRACE"))
    res = run_bass_kernel_spmd(nc, in_maps, list(range(NCORES)), trace=trace)
    global LAST_RESULTS
    LAST_RESULTS = res

    out = np.empty((BATCH, NB * BS), dtype=np.float32)
    for c in range(NCORES):
        outT = res.results[c]["outT"].reshape(NB * BS, BC)
        out[c * BC:(c + 1) * BC, :] = outT.T.astype(np.float32)
    return out

